# revision 5
# baseline (speedup 1.0000x reference)
"""Batch multi-head graph attention (GAT) kernel for 8 Trainium2 NeuronCores.

Reference computation (per batch b, head g):
    hp   = h[b] @ w[g]                        # [N, O]
    t    = tanh(hp)
    src  = t @ a_src[g];  dst = t @ a_dst[g]  # [N]
    s    = leaky_relu(src[q] + dst[m], 0.2)   # [N(q), N(m)]
    e    = exp(s) masked by adj[b][q, m]
    out  = (e @ hp) / rowsum(e) + bias

Device strategy (per core; core c -> b = c//2, heads = (2*(c%2), 2*(c%2)+1)):
  * scores are built TRANSPOSED: tiles [128 m(keys), 2048 q(queries)] so the
    output matmul out^T[o, q] = sum_m hp[m, o] * e[m, q] streams on PE with
    the contraction dim on partitions.  The adjacency mask is pre-transposed
    on the host and streamed in via a dtype-casting DMA (u8 -> bf16).
  * exp factorization: exp(leaky(s)) = max(exp(src+dst), exp(0.2(src+dst)))
                                     = max(A[q]*P[m], B[q]*Q[m])
    with A = exp(src), B = exp(0.2 src) partition-broadcast tiles and
    P = exp(dst[m]), Q = exp(0.2 dst[m]) per-partition scalars; computed with
    one tensor_scalar + one fused scalar_tensor_tensor + one mask multiply.
  * softmax denominator rides as a ones-column in the matmul lhsT; bias is
    folded into hp (softmax weights sum to exactly 1).  Final transpose back
    to [q, o] via PE transpose, then per-row multiply by 1/denominator.
"""

import os
import sys

for _p in ("/opt/trn_rl_repo",):
    if _p not in sys.path and os.path.isdir(_p):
        sys.path.insert(0, _p)

from contextlib import ExitStack

import numpy as np

import concourse.bass as bass
import concourse.tile as tile
from concourse import bacc, mybir
from concourse.bass_utils import run_bass_kernel_spmd

F32 = mybir.dt.float32
BF16 = mybir.dt.bfloat16
U8 = mybir.dt.uint8
AF = mybir.ActivationFunctionType
OP = mybir.AluOpType
AX = mybir.AxisListType

N = 2048          # nodes
F = 256           # input features
O = 64            # output features
P = 128           # partitions
NCH = N // P      # 16 m-chunks per pair
NEG_SLOPE = 0.2
N_CORES = 8
NADJ = 4          # adj is loaded as NADJ independent slabs for overlap

# how many of the 16 m-chunks per pair use the ACT (scalar-engine) pipeline
# (Lrelu+Exp from a raw src broadcast) instead of the DVE factorized pipeline.
ACT_CHUNKS = 0


def _build_pair(nc, pools, consts, hT, adj_slabs, pair, out_d, scratch_h):
    (cpool, epool, pspool) = pools
    ident_sb, ones_sb, w_sb, asrc_sb, adst_sb, bias_sb = consts
    W = O + 1

    # ---- B1: hp = h @ w (+bias, ones col), t = tanh(hp)
    hp_big = cpool.tile([P, NCH * W], BF16, tag="hp_big", name=f"hp{pair}")
    t_cat = cpool.tile([P, NCH * O], F32, tag="t_cat", name=f"tcat{pair}")
    for mc in range(NCH):
        psum_hp = pspool.tile([P, O], F32, tag="hp", bufs=2, name=f"pshp{pair}_{mc}")
        for fc in range(2):
            nc.tensor.matmul(
                psum_hp[:],
                hT[fc][:, mc * P:(mc + 1) * P],
                w_sb[:, (2 * pair + fc) * O:(2 * pair + fc + 1) * O],
                start=(fc == 0),
                stop=(fc == 1),
            )
        nc.vector.tensor_tensor(
            hp_big[:, mc * W: mc * W + O], psum_hp[:], bias_sb[:], OP.add
        )
        nc.scalar.activation(t_cat[:, mc * O:(mc + 1) * O], psum_hp[:], AF.Tanh)
    ones_cols = hp_big.rearrange("p (c k) -> p c k", k=W)[:, :, O:O + 1]
    nc.vector.memset(ones_cols, 1.0)

    # ---- B2: src/dst projections, exp vectors, src broadcast
    prod = cpool.tile([P, NCH * O], F32, tag="prod", name=f"prod{pair}")
    src_col = cpool.tile([P, NCH], F32, tag="src_col", name=f"srcc{pair}")
    dst_col = cpool.tile([P, NCH], F32, tag="dst_col", name=f"dstc{pair}")
    edst = cpool.tile([P, NCH], F32, tag="edst", name=f"edst{pair}")
    edst02 = cpool.tile([P, NCH], F32, tag="edst02", name=f"edst02{pair}")

    nc.vector.tensor_tensor(prod[:], t_cat[:], asrc_sb[:, pair * NCH * O:(pair + 1) * NCH * O], OP.mult)
    nc.vector.tensor_reduce(
        src_col[:], prod.rearrange("p (c k) -> p c k", k=O), AX.X, OP.add
    )
    nc.vector.tensor_tensor(prod[:], t_cat[:], adst_sb[:, pair * NCH * O:(pair + 1) * NCH * O], OP.mult)
    nc.vector.tensor_reduce(
        dst_col[:], prod.rearrange("p (c k) -> p c k", k=O), AX.X, OP.add
    )
    nc.scalar.activation(edst[:], dst_col[:], AF.Exp)
    nc.scalar.activation(edst02[:], dst_col[:], AF.Exp, scale=NEG_SLOPE)

    # src values to DRAM in q-order, then back as a [1, 2048] row
    nc.sync.dma_start(
        bass.AP(scratch_h, pair * N, [[1, P], [P, NCH]]), src_col[:]
    )
    srow = cpool.tile([1, N], F32, tag="srow", name=f"srow{pair}")
    nc.sync.dma_start(srow[:], bass.AP(scratch_h, pair * N, [[N, 1], [1, N]]))

    # broadcast src over partitions via ones outer product, exp on the way out
    psum_bc = pspool.tile([P, N], F32, tag="big", bufs=1, name=f"psbc{pair}")
    for j in range(4):
        nc.tensor.matmul(
            psum_bc[:, j * 512:(j + 1) * 512],
            ones_sb[:],
            srow[:, j * 512:(j + 1) * 512],
            start=True,
            stop=True,
        )
    a_b = cpool.tile([P, N], BF16, tag="a_b", name=f"ab{pair}")
    b_b = cpool.tile([P, N], BF16, tag="b_b", name=f"bb{pair}")
    nc.scalar.activation(a_b[:], psum_bc[:], AF.Exp)
    nc.scalar.activation(b_b[:], psum_bc[:], AF.Exp, scale=NEG_SLOPE)
    src_raw = None
    if ACT_CHUNKS > 0:
        src_raw = cpool.tile([P, N], F32, tag="src_raw", name=f"sraw{pair}")
        nc.vector.tensor_copy(src_raw[:], psum_bc[:])

    # ---- B3: scores + output matmul accumulation over m-chunks
    psum_out = pspool.tile([W, N], F32, tag="big", bufs=1, name=f"psout{pair}")
    for mc in range(NCH):
        slab, qoff = divmod(mc, NCH // NADJ)
        adj_ap = adj_slabs[slab][:, qoff * N:(qoff + 1) * N]
        e_m = epool.tile([P, N], BF16, tag="e_m", name=f"em{pair}_{mc}")
        if mc < NCH - ACT_CHUNKS:
            # DVE pipeline: e = max(A*P[m], B*Q[m]) then mask
            e_neg = epool.tile([P, N], BF16, tag="e_neg", name=f"en{pair}_{mc}")
            e_t = epool.tile([P, N], BF16, tag="e_t", name=f"et{pair}_{mc}")
            nc.vector.tensor_scalar(
                e_neg[:], b_b[:], edst02[:, mc:mc + 1], None, OP.mult
            )
            nc.vector.scalar_tensor_tensor(
                e_t[:], a_b[:], edst[:, mc:mc + 1], e_neg[:], OP.mult, OP.max
            )
            nc.vector.tensor_tensor(e_m[:], e_t[:], adj_ap, OP.mult)
        else:
            # ACT pipeline: s = Lrelu(src + dst[m]); e = Exp(s); mask on DVE
            s_l = epool.tile([P, N], F32, tag="s_l", name=f"sl{pair}_{mc}")
            e_t = epool.tile([P, N], BF16, tag="e_t", name=f"et{pair}_{mc}")
            nc.scalar.activation(
                s_l[:], src_raw[:], AF.Lrelu,
                bias=dst_col[:, mc:mc + 1], alpha=NEG_SLOPE,
            )
            nc.scalar.activation(e_t[:], s_l[:], AF.Exp)
            nc.vector.tensor_tensor(e_m[:], e_t[:], adj_ap, OP.mult)
        for j in range(4):
            nc.tensor.matmul(
                psum_out[:, j * 512:(j + 1) * 512],
                hp_big[:, mc * W:(mc + 1) * W],
                e_m[:, j * 512:(j + 1) * 512],
                start=(mc == 0),
                stop=(mc == NCH - 1),
                skip_group_check=True,
            )

    # ---- B4: transpose back, normalize, store
    outT_sb = cpool.tile([W, N], F32, tag="outT", name=f"outT{pair}")
    nc.vector.tensor_copy(outT_sb[:], psum_out[:])
    out_sb = cpool.tile([P, NCH * O], F32, tag="out_sb", name=f"outsb{pair}")
    rec = cpool.tile([P, NCH], F32, tag="rec", name=f"rec{pair}")
    for qc in range(NCH):
        psum_t = pspool.tile([P, W], F32, tag="tr", bufs=2, name=f"pst{pair}_{qc}")
        nc.tensor.transpose(
            psum_t[:], outT_sb[:, qc * P:(qc + 1) * P], ident_sb[:W, :W]
        )
        nc.vector.reciprocal(rec[:, qc:qc + 1], psum_t[:, O:O + 1])
        nc.vector.tensor_scalar(
            out_sb[:, qc * O:(qc + 1) * O], psum_t[:, :O], rec[:, qc:qc + 1],
            None, OP.mult,
        )
    nc.sync.dma_start(
        out_d[pair].rearrange("(c p) o -> p c o", p=P),
        out_sb.rearrange("p (c k) -> p c k", k=O),
    )


def build_program():
    nc = bacc.Bacc(
        "TRN2",
        target_bir_lowering=False,
        debug=False,
        enable_asserts=True,
        num_devices=1,
    )
    h_d = nc.dram_tensor("h", [N, F], F32, kind="ExternalInput").ap()
    adjt_d = nc.dram_tensor("adjt", [N, N], U8, kind="ExternalInput").ap()
    w_d = nc.dram_tensor("w", [2, F, O], F32, kind="ExternalInput").ap()
    asrcb_d = nc.dram_tensor("asrcb", [2, P, NCH * O], F32, kind="ExternalInput").ap()
    adstb_d = nc.dram_tensor("adstb", [2, P, NCH * O], F32, kind="ExternalInput").ap()
    biasb_d = nc.dram_tensor("biasb", [P, O], F32, kind="ExternalInput").ap()
    ident_d = nc.dram_tensor("ident", [P, P], F32, kind="ExternalInput").ap()
    ones_d = nc.dram_tensor("ones", [1, P], F32, kind="ExternalInput").ap()
    out_d = nc.dram_tensor("out", [2, N, O], F32, kind="ExternalOutput").ap()
    scratch_h = nc.dram_tensor("scratch", [2 * N], F32)

    with tile.TileContext(nc) as tc, ExitStack() as ctx:
        consts_pool = ctx.enter_context(tc.tile_pool(name="consts", bufs=1))
        hpool = ctx.enter_context(tc.tile_pool(name="hpool", bufs=1))
        cpool = ctx.enter_context(tc.tile_pool(name="cpool", bufs=1))
        epool = ctx.enter_context(tc.tile_pool(name="epool", bufs=2))
        pspool = ctx.enter_context(tc.tile_pool(name="psum", bufs=1, space="PSUM"))

        ident_sb = consts_pool.tile([P, P], F32, tag="ident")
        ones_sb = consts_pool.tile([1, P], F32, tag="ones")
        w_sb = consts_pool.tile([P, 2 * 2 * O], F32, tag="w")
        asrc_sb = consts_pool.tile([P, 2 * NCH * O], F32, tag="asrc")
        adst_sb = consts_pool.tile([P, 2 * NCH * O], F32, tag="adst")
        bias_sb = consts_pool.tile([P, O], F32, tag="bias")
        nc.sync.dma_start(ident_sb[:], ident_d[:])
        nc.sync.dma_start(ones_sb[:], ones_d[:])
        nc.sync.dma_start(
            w_sb.rearrange("k (h c o) -> k h c o", h=2, c=2),
            w_d.rearrange("h (c k) o -> k h c o", k=P),
        )
        nc.sync.dma_start(
            asrc_sb.rearrange("p (h n) -> p h n", h=2),
            asrcb_d.rearrange("h p n -> p h n"),
        )
        nc.sync.dma_start(
            adst_sb.rearrange("p (h n) -> p h n", h=2),
            adstb_d.rearrange("h p n -> p h n"),
        )
        nc.sync.dma_start(bias_sb[:], biasb_d[:])

        # adjacency: NADJ independent slabs, u8 -> bf16 cast during DMA
        mper = NCH // NADJ
        adj_slabs = [
            hpool.tile([P, mper * N], BF16, tag=f"adj{s}", name=f"adj{s}")
            for s in range(NADJ)
        ]
        adjt_r = adjt_d.rearrange("(c p) q -> p c q", p=P)  # [128, 16, 2048]
        for s in range(NADJ):
            nc.gpsimd.dma_start(
                adj_slabs[s].rearrange("p (c q) -> p c q", q=N),
                adjt_r[:, s * mper:(s + 1) * mper, :],
            )

        # load h and transpose on PE: hT[fc] is [128 f, 2048 n]
        hN = hpool.tile([P, NCH * F], F32, tag="hN")
        nc.sync.dma_start(
            hN.rearrange("p (c f) -> p c f", f=F),
            h_d.rearrange("(c p) f -> p c f", p=P),
        )
        hT = [
            hpool.tile([P, N], F32, tag=f"hT{fc}", name=f"hT{fc}")
            for fc in range(2)
        ]
        for fc in range(2):
            for mc in range(NCH):
                psum_t = pspool.tile(
                    [P, P], F32, tag="tr", bufs=2, name=f"ht{fc}_{mc}"
                )
                nc.tensor.transpose(
                    psum_t[:], hN[:, mc * F + fc * P: mc * F + (fc + 1) * P],
                    ident_sb[:],
                )
                if (fc * NCH + mc) % 2 == 0:
                    nc.vector.tensor_copy(hT[fc][:, mc * P:(mc + 1) * P], psum_t[:])
                else:
                    nc.scalar.copy(hT[fc][:, mc * P:(mc + 1) * P], psum_t[:])

        pools = (cpool, epool, pspool)
        consts = (ident_sb, ones_sb, w_sb, asrc_sb, adst_sb, bias_sb)
        for pair in range(2):
            _build_pair(nc, pools, consts, hT, adj_slabs, pair, out_d, scratch_h)

    nc.compile()
    return nc


_CACHED = {}


def _get_program():
    if "nc" not in _CACHED:
        _CACHED["nc"] = build_program()
    return _CACHED["nc"]


def make_in_maps(h, adj, w, a_src, a_dst, bias):
    h = np.ascontiguousarray(np.asarray(h, dtype=np.float32))
    adj = np.asarray(adj)
    w = np.asarray(w, dtype=np.float32)
    a_src = np.asarray(a_src, dtype=np.float32).reshape(4, O)
    a_dst = np.asarray(a_dst, dtype=np.float32).reshape(4, O)
    bias = np.asarray(bias, dtype=np.float32).reshape(O)

    adjT = np.ascontiguousarray(adj.transpose(0, 2, 1)).astype(np.uint8)
    biasb = np.ascontiguousarray(np.broadcast_to(bias, (P, O)))
    ident = np.eye(P, dtype=np.float32)
    ones = np.ones((1, P), dtype=np.float32)

    in_maps = []
    for c in range(N_CORES):
        b = c // 2
        hs = [2 * (c % 2), 2 * (c % 2) + 1]
        asrcb = np.ascontiguousarray(
            np.broadcast_to(a_src[hs][:, None, None, :], (2, P, NCH, O))
        ).reshape(2, P, NCH * O)
        adstb = np.ascontiguousarray(
            np.broadcast_to(a_dst[hs][:, None, None, :], (2, P, NCH, O))
        ).reshape(2, P, NCH * O)
        in_maps.append({
            "h": h[b],
            "adjt": adjT[b],
            "w": np.ascontiguousarray(w[hs]),
            "asrcb": asrcb,
            "adstb": adstb,
            "biasb": biasb,
            "ident": ident,
            "ones": ones,
        })
    return in_maps


def assemble(results):
    out = np.empty((4, 4, N, O), dtype=np.float32)
    for c in range(N_CORES):
        b = c // 2
        for i, hd in enumerate((2 * (c % 2), 2 * (c % 2) + 1)):
            out[b, hd] = results[c]["out"][i]
    return out


def kernel(h, adj, w, a_src, a_dst, bias):
    nc = _get_program()
    in_maps = make_in_maps(h, adj, w, a_src, a_dst, bias)
    res = run_bass_kernel_spmd(nc, in_maps, core_ids=list(range(N_CORES)))
    return _assemble_from(res)


def _assemble_from(res):
    return assemble(res.results)
